# revision 12
# baseline (speedup 1.0000x reference)
"""Trainium2 Bass kernel for a 16-head causal MHA layer.

Problem: x:[2,2048,1024] f32, wq/wk/wv/wo:[1024,1024] f32 (Linear-style
[out,in] weights), causal softmax attention with 16 heads of dim 64.

Sharding across the 8 NeuronCores: 2-way data parallel over batch x
4-way tensor parallel over heads.  Core c handles batch c//4 and the 4
heads 4*(c%4) .. 4*(c%4)+3 (feature slice of 256 rows of wq/wk/wv and
256 columns of wo).  Each core produces a partial [2048,1024] output
(its 4 heads' contribution, already projected through its wo slice);
the host sums the 4 partials per batch.

Device dataflow (all matmul inputs fp16, fp32 PSUM accumulation):
  - host uploads x already transposed per batch: xT [1024, 2048] fp16
  - qT/kT = W @ xT in [feat, token] layout; v in [token, feat] layout,
    with a constant-1 column appended per head (v|1)
  - scoresT[k,q] = kT_h.T-block @ qT_h (64-dim contraction), exp on ACT
    straight out of PSUM (no max subtraction: |scores/8| < ~4 so exp is
    safe in fp32/fp16), causal mask applied only on diagonal blocks via
    a precomputed 0/1 mask multiply
  - out_unnorm.T | l = (v|1).T-block @ expT accumulated over k blocks
    (the appended ones-column yields the softmax denominator l for free)
  - 1/l via a DRAM-roundtrip transpose to [128,x] + DVE reciprocal,
    broadcast back across partitions, multiply into out_unnorm.T
  - y = outT.T @ woT accumulated over the 256-dim feature slice
"""

import numpy as np

S = 2048          # sequence length (one batch per core)
D = 1024          # model dim
HL = 4            # heads handled per core
DH = 64           # head dim
F = HL * DH       # 256 local features
DC = D // 128     # 8 d_model chunks of 128
FC = F // 128     # 2 feature chunks of 128
NT = S // 128     # 16 token tiles
NQ = S // 512     # 4 query chunks of 512

_CACHE = {}


def _build_program(dbg=False):
    key = ("nc", dbg)
    if key in _CACHE:
        return _CACHE[key]

    import concourse.bacc as bacc
    import concourse.bass as bass
    import concourse.mybir as mybir
    import concourse.tile as tile

    f16 = mybir.dt.float16
    f32 = mybir.dt.float32
    Exp = mybir.ActivationFunctionType.Exp

    nc = bacc.Bacc("TRN2", target_bir_lowering=False, debug=False)

    xT_d = nc.dram_tensor("xT", [DC, 128, S], f16, kind="ExternalInput")
    wqT_d = nc.dram_tensor("wqT", [DC, 128, F], f16, kind="ExternalInput")
    wkT_d = nc.dram_tensor("wkT", [DC, 128, F], f16, kind="ExternalInput")
    wvT_d = nc.dram_tensor("wvT", [DC, 128, F], f16, kind="ExternalInput")
    woT_d = nc.dram_tensor("woT", [FC, 128, D], f16, kind="ExternalInput")
    mask_d = nc.dram_tensor("mask", [128, 896], f16, kind="ExternalInput")
    y_d = nc.dram_tensor("y", [S, D], f16, kind="ExternalOutput")
    if dbg:
        qT_dbg = nc.dram_tensor("qT_dbg", [128, FC, S], f16, kind="ExternalOutput")
        kT_dbg = nc.dram_tensor("kT_dbg", [128, FC, S], f16, kind="ExternalOutput")
        v_dbg = nc.dram_tensor("v_dbg", [128, NT, HL, DH + 1], f16, kind="ExternalOutput")
        outT_dbg = nc.dram_tensor("outT_dbg", [128, FC, S], f16, kind="ExternalOutput")
        l_dbg = nc.dram_tensor("l_dbg", [HL * S], f32, kind="ExternalOutput")
        rbc_dbg = nc.dram_tensor("rbc_dbg", [128, FC, S], f16, kind="ExternalOutput")
        lt_dbg = nc.dram_tensor("lt_dbg", [128, HL * NT], f32, kind="ExternalOutput")
        rt16_dbg = nc.dram_tensor("rt16_dbg", [128, HL * NT], f16, kind="ExternalOutput")
        rrow_dbg = nc.dram_tensor("rrow_dbg", [HL * S], f16, kind="ExternalOutput")

    with tile.TileContext(nc) as tc:
        with tc.tile_pool(name="const", bufs=1) as cpool, \
             tc.tile_pool(name="dscr", bufs=1,
                          space=bass.MemorySpace.DRAM) as dpool:
            l_dram = dpool.tile([HL * S], f32)
            r_dram = dpool.tile([HL * S], f16)
            xT = cpool.tile([128, DC, S], f16)
            wq = cpool.tile([128, DC, F], f16)
            wk = cpool.tile([128, DC, F], f16)
            wv = cpool.tile([128, DC, F], f16)
            wo = cpool.tile([128, FC, D], f16)
            mask = cpool.tile([128, 896], f16)
            qT = cpool.tile([128, FC, S], f16)
            kT = cpool.tile([128, FC, S], f16)
            v = cpool.tile([128, NT, HL, DH + 1], f16)
            outT = cpool.tile([128, FC, S], f16)
            recip_bc = cpool.tile([128, FC, S], f16)
            l_row = cpool.tile([1, HL * S], f32)
            lT = cpool.tile([128, HL * NT], f32)
            recipT = cpool.tile([128, HL * NT], f32)
            recipT16 = cpool.tile([128, HL * NT], f16)

            nc.sync.dma_start(mask[:], mask_d[:])
            for dc in range(DC):
                nc.sync.dma_start(xT[:, dc, :], xT_d[dc])
                nc.sync.dma_start(wq[:, dc, :], wqT_d[dc])
                nc.sync.dma_start(wk[:, dc, :], wkT_d[dc])
                nc.sync.dma_start(wv[:, dc, :], wvT_d[dc])
            for fc in range(FC):
                nc.sync.dma_start(wo[:, fc, :], woT_d[fc])

            # ones columns for the softmax-denominator trick
            nc.gpsimd.memset(v[:], 1.0)

            # ---- projections -------------------------------------------
            with tc.tile_pool(name="proj_ps", bufs=2,
                              space=bass.MemorySpace.PSUM) as pps:
                for w_sb, dstT in ((wq, qT), (wk, kT)):
                    for fc in range(FC):
                        for t5 in range(NQ):
                            ps = pps.tile([128, 512], f32, tag="proj")
                            for dc in range(DC):
                                nc.tensor.matmul(
                                    ps[:],
                                    w_sb[:, dc, fc * 128:(fc + 1) * 128],
                                    xT[:, dc, t5 * 512:(t5 + 1) * 512],
                                    start=(dc == 0), stop=(dc == DC - 1))
                            nc.vector.tensor_copy(
                                dstT[:, fc, t5 * 512:(t5 + 1) * 512], ps[:])
                for tt in range(NT):
                    psv = pps.tile([128, F], f32, tag="projv")
                    for dc in range(DC):
                        nc.tensor.matmul(
                            psv[:],
                            xT[:, dc, tt * 128:(tt + 1) * 128],
                            wv[:, dc, :],
                            start=(dc == 0), stop=(dc == DC - 1))
                    nc.vector.tensor_copy(
                        v[:, tt, :, 0:DH],
                        psv.rearrange("p (h d) -> p h d", h=HL))

            # ---- attention ---------------------------------------------
            with tc.tile_pool(name="sc_ps", bufs=1,
                              space=bass.MemorySpace.PSUM) as scp, \
                 tc.tile_pool(name="av_ps", bufs=2,
                              space=bass.MemorySpace.PSUM) as avp, \
                 tc.tile_pool(name="p_sb", bufs=3) as ppool:
                for hc in range(FC):
                    for qc in range(NQ):
                        avs = []
                        for hp2 in range(2):
                            av = avp.tile([DH + 1, 512], f32, tag=f"av{hp2}",
                                          name=f"av_{hc}_{qc}_{hp2}")
                            avs.append(av)
                        for g in range(qc + 1):
                            for half in range(2):
                                scs = []
                                for hp2 in range(2):
                                    sc = scp.tile([128, 1024], f32,
                                                  tag=f"sc{hp2}",
                                                  name=f"sc_{hc}_{qc}_{g}_{half}_{hp2}")
                                    scs.append(sc)
                                # interleave the two heads' score matmuls:
                                # they sit on partition rows 0-63 / 64-127 so
                                # the PE runs them as packed row-groups
                                for r2 in range(2):
                                    kb = 4 * g + 2 * half + r2
                                    for hp2 in range(2):
                                        hp = hp2 * 64
                                        nc.tensor.matmul(
                                            scs[hp2][:, r2 * 512:(r2 + 1) * 512],
                                            kT[hp:hp + 64, hc,
                                               kb * 128:(kb + 1) * 128],
                                            qT[hp:hp + 64, hc,
                                               qc * 512:(qc + 1) * 512],
                                            start=True, stop=True)
                                for hp2 in range(2):
                                    h = hc * 2 + hp2
                                    p_sb = ppool.tile([128, 1024], f16,
                                                      tag=f"p{hp2}",
                                                      name=f"p_{hc}_{qc}_{g}_{half}_{hp2}")
                                    nc.scalar.activation(p_sb[:], scs[hp2][:], Exp)
                                    if g == qc:
                                        for r2 in range(2):
                                            r = 2 * half + r2
                                            nc.vector.tensor_mul(
                                                p_sb[:, r2 * 512:(r2 + 1) * 512],
                                                p_sb[:, r2 * 512:(r2 + 1) * 512],
                                                mask[:, 384 - 128 * r:
                                                     896 - 128 * r])
                                    for r2 in range(2):
                                        kb = 4 * g + 2 * half + r2
                                        nc.tensor.matmul(
                                            avs[hp2][:],
                                            v[:, kb, h, :],
                                            p_sb[:, r2 * 512:(r2 + 1) * 512],
                                            start=(kb == 0),
                                            stop=(kb == 4 * qc + 3))
                        for hp2 in range(2):
                            h = hc * 2 + hp2
                            hp = hp2 * 64
                            nc.vector.tensor_copy(
                                outT[hp:hp + 64, hc, qc * 512:(qc + 1) * 512],
                                avs[hp2][0:DH, :])
                            nc.vector.tensor_copy(
                                l_row[0:1, h * S + qc * 512:
                                      h * S + (qc + 1) * 512],
                                avs[hp2][DH:DH + 1, :])
                    # reciprocal of the softmax denominators for this head
                    # pair: DRAM round trip to transpose [1,S] -> [128,NT]
                    for hp2 in range(2):
                        h = hc * 2 + hp2
                        hp = hp2 * 64
                        nc.sync.dma_start(l_dram[h * S:(h + 1) * S],
                                          l_row[0:1, h * S:(h + 1) * S])
                        nc.sync.dma_start(
                            lT[:, h * NT:(h + 1) * NT],
                            l_dram[h * S:(h + 1) * S]
                            .rearrange("(t p) -> p t", p=128))
                        nc.vector.reciprocal(recipT[:, h * NT:(h + 1) * NT],
                                             lT[:, h * NT:(h + 1) * NT])
                        nc.vector.tensor_copy(recipT16[:, h * NT:(h + 1) * NT],
                                              recipT[:, h * NT:(h + 1) * NT])
                        nc.sync.dma_start(
                            r_dram[h * S:(h + 1) * S]
                            .rearrange("(t p) -> p t", p=128),
                            recipT16[:, h * NT:(h + 1) * NT])
                        nc.sync.dma_start(recip_bc[hp:hp + 1, hc, :],
                                          r_dram[h * S:(h + 1) * S])
                        s = 1
                        while s < 64:
                            nc.sync.dma_start(
                                recip_bc[hp + s:hp + 2 * s, hc, :],
                                recip_bc[hp:hp + s, hc, :])
                            s *= 2
                    nc.vector.tensor_mul(outT[:, hc, :], outT[:, hc, :],
                                         recip_bc[:, hc, :])

            if dbg:
                nc.sync.dma_start(qT_dbg[:], qT[:])
                nc.sync.dma_start(kT_dbg[:], kT[:])
                nc.sync.dma_start(v_dbg[:], v[:])
                nc.sync.dma_start(outT_dbg[:], outT[:])
                nc.sync.dma_start(l_dbg[:], l_row[0:1, :])
                nc.sync.dma_start(rbc_dbg[:], recip_bc[:])
                nc.sync.dma_start(lt_dbg[:], lT[:])
                nc.sync.dma_start(rt16_dbg[:], recipT16[:])
                nc.sync.dma_start(rrow_dbg[:], r_dram[:])

            # ---- output projection -------------------------------------
            with tc.tile_pool(name="y_ps", bufs=4,
                              space=bass.MemorySpace.PSUM) as yps_pool, \
                 tc.tile_pool(name="y_sb", bufs=4) as ysb_pool:
                for qt in range(NT):
                    for oc in range(2):
                        yps = yps_pool.tile([128, 512], f32, tag="y",
                                            name=f"yps_{qt}_{oc}")
                        for fc in range(FC):
                            nc.tensor.matmul(
                                yps[:],
                                outT[:, fc, qt * 128:(qt + 1) * 128],
                                wo[:, fc, oc * 512:(oc + 1) * 512],
                                start=(fc == 0), stop=(fc == FC - 1))
                        ysb = ysb_pool.tile([128, 512], f16, tag="ysb",
                                            name=f"ysb_{qt}_{oc}")
                        nc.vector.tensor_copy(ysb[:], yps[:])
                        nc.sync.dma_start(
                            y_d[qt * 128:(qt + 1) * 128,
                                oc * 512:(oc + 1) * 512],
                            ysb[:])

    nc.compile()

    from concourse.bass_interp import get_hw_module
    nc.m = get_hw_module(nc.m)

    _CACHE[key] = nc
    return nc


def _make_mask():
    # mask[p, j] = 1 where (j - p) >= 384; slices of width 512 at offset
    # 384-128*r give the causal mask for a diagonal block at relative
    # position r (k block kb = 4*qc + r vs the 512-wide q chunk qc)
    j = np.arange(896)[None, :]
    p = np.arange(128)[:, None]
    return ((j - p) >= 384).astype(np.float16)


def kernel(x, wq, wk, wv, wo):
    x = np.asarray(x, dtype=np.float32)
    wq = np.asarray(wq, dtype=np.float32)
    wk = np.asarray(wk, dtype=np.float32)
    wv = np.asarray(wv, dtype=np.float32)
    wo = np.asarray(wo, dtype=np.float32)

    from concourse import bass_utils

    nc = _build_program()
    mask = _make_mask()

    in_maps = []
    for c in range(8):
        b = c // 4
        hg = c % 4
        fs = slice(hg * F, (hg + 1) * F)
        xT = np.ascontiguousarray(x[b].T).astype(np.float16).reshape(DC, 128, S)
        wqT = np.ascontiguousarray((wq[fs, :] * 0.125).T).astype(np.float16)
        wkT = np.ascontiguousarray(wk[fs, :].T).astype(np.float16)
        wvT = np.ascontiguousarray(wv[fs, :].T).astype(np.float16)
        woT = np.ascontiguousarray(wo[:, fs].T).astype(np.float16)
        in_maps.append({
            "xT": xT,
            "wqT": wqT.reshape(DC, 128, F),
            "wkT": wkT.reshape(DC, 128, F),
            "wvT": wvT.reshape(DC, 128, F),
            "woT": woT.reshape(FC, 128, D),
            "mask": mask,
        })

    res = bass_utils.run_bass_kernel_spmd(nc, in_maps, core_ids=list(range(8)))
    ys = [res.results[c]["y"].astype(np.float32) for c in range(8)]
    out = np.stack([ys[0] + ys[1] + ys[2] + ys[3],
                    ys[4] + ys[5] + ys[6] + ys[7]])
    return out
